# revision 27
# baseline (speedup 1.0000x reference)
"""Multi-head attention V2 kernel for Trainium2 (8 NeuronCores).

Problem shapes (hardcoded): x [4, 2048, 512] f32, Wq [512, 4096], Wv unused,
Wp [4096, 512], bp [512].  Reference math (note: V uses the Q projection):
    q = v = (x @ Wq) -> [B, H, N, D] with H=8, head dim = D = 512
    S = q @ x^T / sqrt(D);  P = softmax(S, -1);  out = (P @ v) @ Wp + bp

Sharding: core = (batch b, head-group hg) with 2 groups of 4 heads.
Each core gets x[b]^T and the Wq columns / Wp rows of its 4 heads, computes
its per-head partial outputs [HPG, N, D]; host sums the 8 head partials per
batch and adds the bias.

Per-core kernel (software-pipelined over 16 (head, chunk) jobs):
  xT [512, 2048] fp16 + fp8e4m3, Wq fp16, Wp fp16 resident in SBUF.
  Stage B per head (emitted during the previous head's jobs):
    q [m, j] = x Wq_h  (fp16 matmuls, fp32 PSUM) -> SBUF fp16 qn
    qT[j, n] = q_h^T via DMA xbar transposes, cast fp16 -> fp8e4m3
               (alternating ScalarE/DVE per 512-column chunk)
  Per job J = (head h, 512-column chunk c):
    S^T[m, n] of job J+1 = x q^T as fp8e4m3 DoubleRow matmuls (2 passes
      of K=256, lhsT [128,2,128] / rhs [128,2,512], same rows/cycle as
      fp16 -> 2x FLOP rate), INTERLEAVED with job J's AV-dt0 matmuls so
      the PE (647ns/iter) outpaces ScalarE's exp (667ns) without
      stalling on the scores-PSUM ring.  fp8 scores cost ~1.9e-2 global
      rel err (fp16 elsewhere keeps the rest at ~4e-4), under the 2e-2
      gate.  expS = exp(S^T/sqrt(D)) on ScalarE, PSUM -> SBUF fp16.
    den: DVE pairwise tree-add of J's 16 expS tiles (4 wide fp16 adds,
      emitted ahead of stage-B's qn copies in the DVE queue), then ONE
      all-ones [128,128] fp16 matmul that simultaneously sums over the
      128 partitions and broadcasts den to all partitions.
    rcpB = 1/den (DVE reciprocal_approx_fast); U^T = q^T expS (fp16
      matmuls, lhsT=qn); outT = U^T * rcpB (DVE, muls staggered one dt
      late so the reciprocal chain has PE cover).
    y_h[n, e] = sum_dt outT[dt, n-tile]^T Wp[h*4+dt] -> HBM f32 partial
      per head; the host sums the 8 per-head partials per batch + bias.
Softmax skips the max-subtraction: scores are q.x/sqrt(512) with |s| < ~6,
so exp is safely in fp32 range and the result is mathematically identical.
"""

import os
import sys

sys.path.insert(0, "/opt/trn_rl_repo")

import numpy as np
import ml_dtypes

# fp8 DoubleRow scores toggle (KFP8=0 falls back to fp16 scores, for
# debugging accuracy vs speed)
FP8_S = os.environ.get("KFP8", "1") == "1"

B, N, D, H = 4, 2048, 512, 8
NCORES = 8
HG = 2            # head groups (cores per batch)
HPG = H // HG     # heads per core
JW = HPG * D      # per-core Wq column count / Wp row count (2048)
KT = D // 128     # k-tiles over feature dim (4)
NT = N // 128     # partition tiles over tokens (16)
NCHUNK = 4        # n split into 4 chunks of 512
CW = N // NCHUNK  # chunk width (512)
INV_SQRT_D = 1.0 / float(np.sqrt(D))

_state = {}


def _build():
    import concourse.bass as bass
    import concourse.mybir as mybir
    import concourse.tile as tile
    from concourse import bacc

    f32 = mybir.dt.float32
    bf16 = mybir.dt.float16
    fp8 = mybir.dt.float8e4
    DR = mybir.MatmulPerfMode.DoubleRow

    nc = bacc.Bacc("TRN2", target_bir_lowering=False)

    xT_d = nc.dram_tensor("xt", [D, N], bf16, kind="ExternalInput")
    x8_d = (nc.dram_tensor("xt8", [D, N], fp8, kind="ExternalInput")
            if FP8_S else None)
    wq_d = nc.dram_tensor("wq", [D, JW], bf16, kind="ExternalInput")
    wp_d = nc.dram_tensor("wp", [JW, D], bf16, kind="ExternalInput")
    y_d = nc.dram_tensor("y", [HPG * N, D], f32, kind="ExternalOutput")

    with tile.TileContext(nc) as tc:
        with (
            tc.tile_pool(name="const", bufs=1) as cpool,
            tc.tile_pool(name="qt16", bufs=1) as qt16_pool,
            tc.tile_pool(name="qt8", bufs=2) as qt8_pool,
            tc.tile_pool(name="qn", bufs=2) as qn_pool,
            tc.tile_pool(name="exps", bufs=2) as exps_pool,
            tc.tile_pool(name="dtree", bufs=2) as dtree_pool,
            tc.tile_pool(name="outt", bufs=2) as outt_pool,
            tc.tile_pool(name="rcp", bufs=2) as rcp_pool,
            tc.tile_pool(name="ysb", bufs=3) as y_pool,
            tc.tile_pool(name="ps_stage", bufs=2, space="PSUM") as ps_stage,
            tc.tile_pool(name="ps_scores", bufs=3, space="PSUM") as ps_scores,
            tc.tile_pool(name="ps_av", bufs=2, space="PSUM") as ps_av,
            tc.tile_pool(name="ps_misc", bufs=1, space="PSUM") as ps_misc,
        ):
            # ---- resident inputs ----
            xT = cpool.tile([128, KT, N], bf16, name="xT")
            x8 = (cpool.tile([128, KT, N], fp8, name="x8")
                  if FP8_S else None)
            wq = cpool.tile([128, KT, JW], bf16, name="wq")
            wp = cpool.tile([128, JW // 128, D], bf16, name="wp")
            # critical first wave, finest first: the very first stage-B
            # matmul group needs only xT cols 0:128 of each k-tile plus the
            # head-0 Wq block (~640KB), so land those before the rest
            for k in range(KT):
                nc.sync.dma_start(
                    xT[:, k, 0:128], xT_d[k * 128 : (k + 1) * 128, 0:128]
                )
                # split the head-0 wq tiles so no single queue serializes
                # the very first stage-B matmul group
                for w in range(2):
                    nc.sync.dma_start(
                        wq[:, k, w * 256 : (w + 1) * 256],
                        wq_d[k * 128 : (k + 1) * 128, w * 256 : (w + 1) * 256],
                    )
            for k in range(KT):
                nc.sync.dma_start(
                    xT[:, k, 128:CW], xT_d[k * 128 : (k + 1) * 128, 128:CW]
                )
            # xT tail in 256-col blocks: b_tile(mt) only needs its own
            # column block of every k-row, so finer transfers unblock the
            # stage-B pipeline several us earlier
            for c0 in range(CW, N, 256):
                for k in range(KT):
                    nc.sync.dma_start(
                        xT[:, k, c0 : c0 + 256],
                        xT_d[k * 128 : (k + 1) * 128, c0 : c0 + 256],
                    )
            # fp8 copy of x^T, also in column blocks (prologue scores tile
            # mt needs only column block mt of all four k-rows)
            if FP8_S:
                for c0 in range(0, N, 512):
                    for k in range(KT):
                        nc.sync.dma_start(
                            x8[:, k, c0 : c0 + 512],
                            x8_d[k * 128 : (k + 1) * 128, c0 : c0 + 512],
                        )

            def load_noncritical():
                # wq for heads 1-3 (first needed ~100us in) and wp (first
                # needed by head-0 chunk-0 projection): emitted after head
                # 0's transposes so the critical wave gets full bandwidth
                for h in range(1, HPG):
                    for k in range(KT):
                        nc.sync.dma_start(
                            wq[:, k, h * D : (h + 1) * D],
                            wq_d[k * 128 : (k + 1) * 128, h * D : (h + 1) * D],
                        )
                for j in range(JW // 128):
                    nc.sync.dma_start(wp[:, j, :], wp_d[j * 128 : (j + 1) * 128, :])

            load_noncritical()

            ones_col = cpool.tile([128, 1], bf16, name="ones_col")
            nc.vector.memset(ones_col[:, :], 1.0)
            # touch Exp once during the input-DMA wait so the ~2.7us ACT
            # table-set load is off the first chunk's critical path
            nc.scalar.activation(
                ones_col[:, :], ones_col[:, :],
                mybir.ActivationFunctionType.Exp, scale=0.0,
            )
            nc.vector.memset(ones_col[:, :], 1.0)
            # all-ones [128,128] fp16: one matmul against the tree-reduced
            # dtree row block both sums over partitions and broadcasts the
            # result to all 128 output partitions
            ones128 = cpool.tile([128, 128], bf16, name="ones128")
            nc.vector.memset(ones128[:, :], 1.0)

            def emit_stage_b(h, part=0, tiles=None):
                # stage B: q_h [m, j] (token-major); qT via DMA xbar, then
                # cast to fp8 per 512-col chunk on alternating ScalarE/DVE.
                # For h>0 emitted in two halves (part 0: token tiles 0-7 +
                # casts c0/c1, part 1: tiles 8-15 + casts c2/c3) so each
                # half's qn copies drain within one job's DVE window.
                j0 = h * D
                if tiles is None:
                    qT = qt16_pool.tile([128, KT, N], bf16, name="qT", tag="qT")
                    qT8 = (qt8_pool.tile([128, KT, N], fp8, name="qT8",
                                         tag="qT8") if FP8_S else None)
                    qn = qn_pool.tile([128, NT, D], bf16, name="qn", tag="qn")
                else:
                    qT, qT8, qn = tiles

                def b_tile(mt):
                    ps = ps_stage.tile([128, D], f32, name="ps_b", tag="stage")
                    for k in range(KT):
                        nc.tensor.matmul(
                            ps[:, :],
                            lhsT=xT[:, k, mt * 128 : (mt + 1) * 128],
                            rhs=wq[:, k, j0 : j0 + D],
                            start=(k == 0),
                            stop=(k == KT - 1),
                        )
                    nc.vector.tensor_copy(qn[:, mt, :], ps[:, :])
                    # one xbar transpose per mt: [128, 512] -> [512, 128]
                    # scattered over the 4 j-tiles of qT (3D dest AP)
                    if h != 0 or mt >= CW // 128:
                        nc.sync.dma_start_transpose(
                            qT[:, :, mt * 128 : (mt + 1) * 128], qn[:, mt, :]
                        )

                def cast_chunk(c):
                    if not FP8_S:
                        return
                    # all casts on ScalarE: it is idle in the stage-B window
                    # (exps ran during the interleave), while DVE casts would
                    # delay the normalize-muls behind the qn copies
                    nc.scalar.copy(
                        qT8[:, :, c * CW : (c + 1) * CW],
                        qT[:, :, c * CW : (c + 1) * CW],
                    )

                if h == 0 and part == 0:
                    # head 0 has no prior work to hide the transpose latency
                    # behind: compute its first qT chunk directly on the PE,
                    # writing fp8 straight from PSUM via ScalarE.  Token
                    # tiles 12-15 are deferred to part 1, emitted mid-
                    # prologue to fill the PE idle behind the prologue exps.
                    for mt in range(4):
                        b_tile(mt)
                    for jt in range(KT):
                        ps = ps_stage.tile([128, CW], f32, name="ps_a", tag="stage")
                        for k in range(KT):
                            nc.tensor.matmul(
                                ps[:, :],
                                lhsT=wq[:, k, jt * 128 : (jt + 1) * 128],
                                rhs=xT[:, k, 0:CW],
                                start=(k == 0),
                                stop=(k == KT - 1),
                            )
                        nc.scalar.copy(
                            (qT8 if FP8_S else qT)[:, jt, 0:CW], ps[:, :]
                        )
                    for c in range(1, NCHUNK - 1):
                        for mt in range(4 * c, 4 * c + 4):
                            b_tile(mt)
                        cast_chunk(c)
                elif h == 0:
                    for mt in range(12, 16):
                        b_tile(mt)
                    cast_chunk(NCHUNK - 1)
                else:
                    for c in range(2 * part, 2 * part + 2):
                        for mt in range(4 * c, 4 * c + 4):
                            b_tile(mt)
                        cast_chunk(c)
                return (qT, qT8, qn)

            def emit_S(qT8s, c, mt, ps):
                # scores matmuls for (chunk c, token tile mt) into PSUM ps
                n0 = c * CW
                if FP8_S:
                    for kp in range(KT // 2):
                        nc.tensor.matmul(
                            ps[:, :],
                            lhsT=x8[:, 2 * kp : 2 * kp + 2,
                                    mt * 128 : (mt + 1) * 128],
                            rhs=qT8s[:, 2 * kp : 2 * kp + 2, n0 : n0 + CW],
                            start=(kp == 0),
                            stop=(kp == KT // 2 - 1),
                            perf_mode=DR,
                        )
                else:
                    for k in range(KT):
                        nc.tensor.matmul(
                            ps[:, :],
                            lhsT=xT[:, k, mt * 128 : (mt + 1) * 128],
                            rhs=qT8s[:, k, n0 : n0 + CW],
                            start=(k == 0),
                            stop=(k == KT - 1),
                        )

            # ---- software-pipelined main loop over 16 (head, chunk) jobs.
            # Job J's AV-dt0 matmuls interleave with job J+1's scores
            # matmuls so the PE never drains behind ScalarE's exp (600ns /
            # 512-col tile vs 434ns of fp8 scores matmul work). ----
            jobs = [(h, c) for h in range(HPG) for c in range(NCHUNK)]
            pending = emit_stage_b(0)
            _, qT8_cur, qn_cur = (pending[0], pending[1] or pending[0],
                                  pending[2])

            # prologue: job (0,0)'s scores, un-interleaved (overlaps the
            # input DMA waits at kernel start); head 0's deferred stage-B
            # tail fills the PE idle behind the prologue's exps
            expS_cur = exps_pool.tile([128, NT, CW], bf16, name="expS",
                                      tag="expS")
            for mt in range(NT):
                if mt == 8:
                    pending = emit_stage_b(0, part=1, tiles=pending)
                ps = ps_scores.tile([128, CW], f32, name="ps_s", tag="scores")
                emit_S(qT8_cur, 0, mt, ps)
                nc.scalar.activation(
                    expS_cur[:, mt, :], ps[:, :],
                    mybir.ActivationFunctionType.Exp, scale=INV_SQRT_D,
                )

            dtree_cur = dtree_pool.tile([128, 8, CW], bf16, name="dtree",
                                         tag="dtree")
            nc.vector.tensor_add(
                dtree_cur[:, 0:8, :], expS_cur[:, 0:8, :], expS_cur[:, 8:16, :]
            )
            nc.vector.tensor_add(
                dtree_cur[:, 0:4, :], dtree_cur[:, 0:4, :], dtree_cur[:, 4:8, :]
            )
            nc.vector.tensor_add(
                dtree_cur[:, 0:2, :], dtree_cur[:, 0:2, :], dtree_cur[:, 2:4, :]
            )
            nc.vector.tensor_add(
                dtree_cur[:, 0:1, :], dtree_cur[:, 0:1, :], dtree_cur[:, 1:2, :]
            )

            for idx, (h, c) in enumerate(jobs):
                nxt = jobs[idx + 1] if idx + 1 < len(jobs) else None
                if nxt is not None:
                    nh, ncc = nxt
                    if ncc == 0:
                        qT8_nxt = pending[1] if FP8_S else pending[0]
                        qn_nxt = pending[2]
                    else:
                        qT8_nxt, qn_nxt = qT8_cur, qn_cur
                    expS_nxt = exps_pool.tile([128, NT, CW], bf16,
                                              name="expS", tag="expS")

                # AV dt0 accumulation, interleaved with next job's scores
                outT = outt_pool.tile([128, KT, CW], bf16, name="outT",
                                      tag="outT")
                ps0 = ps_av.tile([128, CW], f32, name="ps_av", tag="av")
                for mt in range(NT):
                    if nxt is not None:
                        pss = ps_scores.tile([128, CW], f32, name="ps_s",
                                             tag="scores")
                        emit_S(qT8_nxt, ncc, mt, pss)
                        nc.scalar.activation(
                            expS_nxt[:, mt, :], pss[:, :],
                            mybir.ActivationFunctionType.Exp,
                            scale=INV_SQRT_D,
                        )
                    nc.tensor.matmul(
                        ps0[:, :],
                        lhsT=qn_cur[:, mt, 0:128],
                        rhs=expS_cur[:, mt, :],
                        start=(mt == 0),
                        stop=(mt == NT - 1),
                    )

                # single sum+broadcast matmul for the denominator (emitted
                # after the loop: earlier would head-of-line block the PE
                # on the DVE tree), then the reciprocal
                psb = ps_misc.tile([128, CW], f32, name="psb", tag="misc")
                nc.tensor.matmul(
                    psb[:, :], lhsT=ones128[:, :], rhs=dtree_cur[:, 0, :],
                    start=True, stop=True,
                )
                rcpB = rcp_pool.tile([128, CW], f32, name="rcpB", tag="rcpB")
                nc.vector.reciprocal_approx_fast(rcpB[:, :], psb[:, :])

                # next job's denominator tree, emitted now so it sits ahead
                # of stage-B's qn copies in the DVE queue
                if nxt is not None:
                    dtree_nxt = dtree_pool.tile([128, 8, CW], bf16,
                                                name="dtree", tag="dtree")
                    nc.vector.tensor_add(
                        dtree_nxt[:, 0:8, :], expS_nxt[:, 0:8, :],
                        expS_nxt[:, 8:16, :]
                    )
                    nc.vector.tensor_add(
                        dtree_nxt[:, 0:4, :], dtree_nxt[:, 0:4, :],
                        dtree_nxt[:, 4:8, :]
                    )
                    nc.vector.tensor_add(
                        dtree_nxt[:, 0:2, :], dtree_nxt[:, 0:2, :],
                        dtree_nxt[:, 2:4, :]
                    )
                    nc.vector.tensor_add(
                        dtree_nxt[:, 0:1, :], dtree_nxt[:, 0:1, :],
                        dtree_nxt[:, 1:2, :]
                    )

                # next head's stage B (full), before the dt1-3 block
                if c == NCHUNK - 2 and h + 1 < HPG:
                    pending = emit_stage_b(h + 1, part=0)
                    pending = emit_stage_b(h + 1, part=1, tiles=pending)

                # AV dt1..3; each dt's normalize-mul is emitted one dt late
                # so the reciprocal chain has a full dt of PE cover
                for dt in range(1, KT):
                    ps = ps_av.tile([128, CW], f32, name="ps_av", tag="av")
                    for mt in range(NT):
                        nc.tensor.matmul(
                            ps[:, :],
                            lhsT=qn_cur[:, mt, dt * 128 : (dt + 1) * 128],
                            rhs=expS_cur[:, mt, :],
                            start=(mt == 0),
                            stop=(mt == NT - 1),
                        )
                    prev = ps0 if dt == 1 else prev_ps
                    nc.vector.tensor_mul(
                        outT[:, dt - 1, :], prev[:, :], rcpB[:, :]
                    )
                    prev_ps = ps
                nc.vector.tensor_mul(
                    outT[:, KT - 1, :], prev_ps[:, :], rcpB[:, :]
                )

                # per-head projection for this chunk's n-tiles; host sums
                # the head partials
                def y_emit(ntl, dts, ps=None):
                    if ps is None:
                        ps = ps_stage.tile([128, D], f32, name="ps_y",
                                           tag="stage")
                    for dt in dts:
                        nc.tensor.matmul(
                            ps[:, :],
                            lhsT=outT[:, dt, ntl * 128 : (ntl + 1) * 128],
                            rhs=wp[:, h * KT + dt, :],
                            start=(dt == 0),
                            stop=(dt == KT - 1),
                        )
                    if dts[-1] != KT - 1:
                        return ps
                    nt = c * (CW // 128) + ntl
                    ysb = y_pool.tile([128, D], f32, name="ysb", tag="y")
                    if nt % 2 == 0:
                        nc.scalar.copy(ysb[:, :], ps[:, :])
                    else:
                        nc.vector.tensor_copy(ysb[:, :], ps[:, :])
                    nc.sync.dma_start(
                        y_d[h * N + nt * 128 : h * N + (nt + 1) * 128, :],
                        ysb[:, :],
                    )
                    return None

                # first two n-tiles defer their dt3 matmul so the PE isn't
                # waiting on the last normalize-mul right after dt3's
                # accumulation finishes
                p0 = y_emit(0, [0, 1, 2])
                p1 = y_emit(1, [0, 1, 2])
                y_emit(0, [3], ps=p0)
                y_emit(1, [3], ps=p1)
                y_emit(2, [0, 1, 2, 3])
                y_emit(3, [0, 1, 2, 3])

                if nxt is not None:
                    expS_cur = expS_nxt
                    dtree_cur = dtree_nxt
                    qT8_cur, qn_cur = qT8_nxt, qn_nxt

    nc.compile()
    return nc


def _ensure_nc():
    if "nc" not in _state:
        _state["nc"] = _build()
    return _state["nc"]


def _make_in_maps(x, Wq, Wp):
    bf = np.float16
    f8 = ml_dtypes.float8_e4m3
    in_maps = []
    for c in range(NCORES):
        b, hg = c // HG, c % HG
        xt = np.ascontiguousarray(x[b].T)
        m = {
            "xt": xt.astype(bf),
            "wq": np.ascontiguousarray(Wq[:, hg * JW : (hg + 1) * JW]).astype(bf),
            "wp": np.ascontiguousarray(Wp[hg * JW : (hg + 1) * JW, :]).astype(bf),
        }
        if FP8_S:
            m["xt8"] = xt.astype(f8)
        in_maps.append(m)
    return in_maps


def _get_runner():
    """Build once and cache a jitted 8-core runner (avoids re-jit per call)."""
    if "run" in _state:
        return _state["run"]

    import jax
    import concourse.mybir as mybir
    from jax.sharding import Mesh, PartitionSpec
    from jax.experimental.shard_map import shard_map
    from concourse import bass2jax

    nc = _ensure_nc()
    bass2jax.install_neuronx_cc_hook()

    partition_name = nc.partition_id_tensor.name if nc.partition_id_tensor else None
    in_names, out_names, out_avals, zero_outs = [], [], [], []
    for alloc in nc.m.functions[0].allocations:
        if not isinstance(alloc, mybir.MemoryLocationSet):
            continue
        name = alloc.memorylocations[0].name
        if alloc.kind == "ExternalInput":
            if name != partition_name:
                in_names.append(name)
        elif alloc.kind == "ExternalOutput":
            shape = tuple(alloc.tensor_shape)
            dtype = mybir.dt.np(alloc.dtype)
            out_avals.append(jax.core.ShapedArray(shape, dtype))
            out_names.append(name)
            zero_outs.append(np.zeros(shape, dtype))
    n_params = len(in_names)
    n_outs = len(out_names)
    all_in_names = list(in_names) + list(out_names)
    if partition_name is not None:
        all_in_names.append(partition_name)

    def _body(*args):
        operands = list(args)
        if partition_name is not None:
            operands.append(bass2jax.partition_id_tensor())
        outs = bass2jax._bass_exec_p.bind(
            *operands,
            out_avals=tuple(out_avals),
            in_names=tuple(all_in_names),
            out_names=tuple(out_names),
            lowering_input_output_aliases=(),
            sim_require_finite=True,
            sim_require_nnan=True,
            nc=nc,
        )
        return tuple(outs)

    devices = jax.devices()[:NCORES]
    mesh = Mesh(np.asarray(devices), ("core",))
    in_specs = (PartitionSpec("core"),) * (n_params + n_outs)
    out_specs = (PartitionSpec("core"),) * n_outs
    sharded = jax.jit(
        shard_map(_body, mesh=mesh, in_specs=in_specs, out_specs=out_specs,
                  check_rep=False),
        donate_argnums=tuple(range(n_params, n_params + n_outs)),
        keep_unused=True,
    )

    def run(in_maps):
        concat_in = [
            np.concatenate([np.asarray(m[name]) for m in in_maps], axis=0)
            for name in in_names
        ]
        concat_zeros = [
            np.zeros((NCORES * z.shape[0], *z.shape[1:]), z.dtype) for z in zero_outs
        ]
        out_arrs = sharded(*concat_in, *concat_zeros)
        return [
            {
                name: np.asarray(out_arrs[i]).reshape(NCORES, *out_avals[i].shape)[c]
                for i, name in enumerate(out_names)
            }
            for c in range(NCORES)
        ]

    _state["run"] = run
    return run


def kernel(x, Wq, Wv, Wp, bp):
    x = np.asarray(x, np.float32)
    Wq = np.asarray(Wq, np.float32)
    Wp = np.asarray(Wp, np.float32)
    bp = np.asarray(bp, np.float32)

    run = _get_runner()
    results = run(_make_in_maps(x, Wq, Wp))
    y = np.empty((B, N, D), np.float32)
    for b in range(B):
        y[b] = (results[b * HG]["y"].reshape(HPG, N, D).sum(axis=0)
                + results[b * HG + 1]["y"].reshape(HPG, N, D).sum(axis=0)
                + bp[None, :])
    return y


# revision 28
# speedup vs baseline: 1.0387x; 1.0387x over previous
"""Multi-head attention V2 kernel for Trainium2 (8 NeuronCores).

Problem shapes (hardcoded): x [4, 2048, 512] f32, Wq [512, 4096], Wv unused,
Wp [4096, 512], bp [512].  Reference math (note: V uses the Q projection):
    q = v = (x @ Wq) -> [B, H, N, D] with H=8, head dim = D = 512
    S = q @ x^T / sqrt(D);  P = softmax(S, -1);  out = (P @ v) @ Wp + bp

Sharding: core = (batch b, head-group hg) with 2 groups of 4 heads.
Each core gets x[b]^T and the Wq columns / Wp rows of its 4 heads, computes
its per-head partial outputs [HPG, N, D]; host sums the 8 head partials per
batch and adds the bias.

Per-core kernel (software-pipelined over 16 (head, chunk) jobs):
  xT [512, 2048] fp16 + fp8e4m3, Wq fp16, Wp fp16 resident in SBUF.
  Stage B per head (emitted during the previous head's jobs):
    q [m, j] = x Wq_h  (fp16 matmuls, fp32 PSUM) -> SBUF fp16 qn
    qT[j, n] = q_h^T via DMA xbar transposes, cast fp16 -> fp8e4m3
               (alternating ScalarE/DVE per 512-column chunk)
  Per job J = (head h, 512-column chunk c):
    S^T[m, n] of job J+1 = x q^T as fp8e4m3 DoubleRow matmuls (2 passes
      of K=256, lhsT [128,2,128] / rhs [128,2,512], same rows/cycle as
      fp16 -> 2x FLOP rate), INTERLEAVED with job J's AV-dt0 matmuls so
      the PE (647ns/iter) outpaces ScalarE's exp (667ns) without
      stalling on the scores-PSUM ring.  fp8 scores cost ~1.9e-2 global
      rel err (fp16 elsewhere keeps the rest at ~4e-4), under the 2e-2
      gate.  expS = exp(S^T/sqrt(D)) on ScalarE, PSUM -> SBUF fp16.
    den: DVE pairwise tree-add of J's 16 expS tiles (4 wide fp16 adds,
      emitted ahead of stage-B's qn copies in the DVE queue), then ONE
      all-ones [128,128] fp16 matmul that simultaneously sums over the
      128 partitions and broadcasts den to all partitions.
    rcpB = 1/den (DVE reciprocal_approx_fast); U^T = q^T expS (fp16
      matmuls, lhsT=qn); outT = U^T * rcpB (DVE, muls staggered one dt
      late so the reciprocal chain has PE cover).
    y_h[n, e] = sum_dt outT[dt, n-tile]^T Wp[h*4+dt] -> HBM f32 partial
      per head; the host sums the 8 per-head partials per batch + bias.
Softmax skips the max-subtraction: scores are q.x/sqrt(512) with |s| < ~6,
so exp is safely in fp32 range and the result is mathematically identical.
"""

import os
import sys

sys.path.insert(0, "/opt/trn_rl_repo")

import numpy as np
import ml_dtypes

# fp8 DoubleRow scores toggle (KFP8=0 falls back to fp16 scores, for
# debugging accuracy vs speed)
FP8_S = os.environ.get("KFP8", "1") == "1"

B, N, D, H = 4, 2048, 512, 8
NCORES = 8
HG = 2            # head groups (cores per batch)
HPG = H // HG     # heads per core
JW = HPG * D      # per-core Wq column count / Wp row count (2048)
KT = D // 128     # k-tiles over feature dim (4)
NT = N // 128     # partition tiles over tokens (16)
NCHUNK = 4        # n split into 4 chunks of 512
CW = N // NCHUNK  # chunk width (512)
INV_SQRT_D = 1.0 / float(np.sqrt(D))

_state = {}


def _build():
    import concourse.bass as bass
    import concourse.mybir as mybir
    import concourse.tile as tile
    from concourse import bacc

    f32 = mybir.dt.float32
    bf16 = mybir.dt.float16
    fp8 = mybir.dt.float8e4
    DR = mybir.MatmulPerfMode.DoubleRow

    nc = bacc.Bacc("TRN2", target_bir_lowering=False)

    xT_d = nc.dram_tensor("xt", [D, N], bf16, kind="ExternalInput")
    x8_d = (nc.dram_tensor("xt8", [D, N], fp8, kind="ExternalInput")
            if FP8_S else None)
    wq_d = nc.dram_tensor("wq", [D, JW], bf16, kind="ExternalInput")
    wp_d = nc.dram_tensor("wp", [JW, D], bf16, kind="ExternalInput")
    y_d = nc.dram_tensor("y", [HPG * N, D], f32, kind="ExternalOutput")

    with tile.TileContext(nc) as tc:
        with (
            tc.tile_pool(name="const", bufs=1) as cpool,
            tc.tile_pool(name="qt16", bufs=1) as qt16_pool,
            tc.tile_pool(name="qt8", bufs=2) as qt8_pool,
            tc.tile_pool(name="qn", bufs=2) as qn_pool,
            tc.tile_pool(name="exps", bufs=2) as exps_pool,
            tc.tile_pool(name="dtree", bufs=2) as dtree_pool,
            tc.tile_pool(name="outt", bufs=2) as outt_pool,
            tc.tile_pool(name="rcp", bufs=2) as rcp_pool,
            tc.tile_pool(name="ysb", bufs=3) as y_pool,
            tc.tile_pool(name="ps_stage", bufs=2, space="PSUM") as ps_stage,
            tc.tile_pool(name="ps_scores", bufs=3, space="PSUM") as ps_scores,
            tc.tile_pool(name="ps_av", bufs=2, space="PSUM") as ps_av,
            tc.tile_pool(name="ps_misc", bufs=1, space="PSUM") as ps_misc,
        ):
            # ---- resident inputs ----
            xT = cpool.tile([128, KT, N], bf16, name="xT")
            x8 = (cpool.tile([128, KT, N], fp8, name="x8")
                  if FP8_S else None)
            wq = cpool.tile([128, KT, JW], bf16, name="wq")
            wp = cpool.tile([128, JW // 128, D], bf16, name="wp")
            # critical first wave, finest first: the very first stage-B
            # matmul group needs only xT cols 0:128 of each k-tile plus the
            # head-0 Wq block (~640KB), so land those before the rest
            for k in range(KT):
                nc.sync.dma_start(
                    xT[:, k, 0:128], xT_d[k * 128 : (k + 1) * 128, 0:128]
                )
                # split the head-0 wq tiles so no single queue serializes
                # the very first stage-B matmul group
                for w in range(2):
                    nc.sync.dma_start(
                        wq[:, k, w * 256 : (w + 1) * 256],
                        wq_d[k * 128 : (k + 1) * 128, w * 256 : (w + 1) * 256],
                    )
            for k in range(KT):
                nc.sync.dma_start(
                    xT[:, k, 128:CW], xT_d[k * 128 : (k + 1) * 128, 128:CW]
                )
            for k in range(KT):
                nc.sync.dma_start(
                    xT[:, k, CW:N], xT_d[k * 128 : (k + 1) * 128, CW:N]
                )
            # fp8 copy of x^T: first consumed by head-0 chunk-0 scores,
            # ~25us in; lands after the fp16 critical waves
            if FP8_S:
                for k in range(KT):
                    nc.sync.dma_start(
                        x8[:, k, :], x8_d[k * 128 : (k + 1) * 128, :]
                    )

            def load_noncritical():
                # wq for heads 1-3 (first needed ~100us in) and wp (first
                # needed by head-0 chunk-0 projection): emitted after head
                # 0's transposes so the critical wave gets full bandwidth
                for h in range(1, HPG):
                    for k in range(KT):
                        nc.sync.dma_start(
                            wq[:, k, h * D : (h + 1) * D],
                            wq_d[k * 128 : (k + 1) * 128, h * D : (h + 1) * D],
                        )
                for j in range(JW // 128):
                    nc.sync.dma_start(wp[:, j, :], wp_d[j * 128 : (j + 1) * 128, :])

            load_noncritical()

            ones_col = cpool.tile([128, 1], bf16, name="ones_col")
            nc.vector.memset(ones_col[:, :], 1.0)
            # touch Exp once during the input-DMA wait so the ~2.7us ACT
            # table-set load is off the first chunk's critical path
            nc.scalar.activation(
                ones_col[:, :], ones_col[:, :],
                mybir.ActivationFunctionType.Exp, scale=0.0,
            )
            nc.vector.memset(ones_col[:, :], 1.0)
            # all-ones [128,128] fp16: one matmul against the tree-reduced
            # dtree row block both sums over partitions and broadcasts the
            # result to all 128 output partitions
            ones128 = cpool.tile([128, 128], bf16, name="ones128")
            nc.vector.memset(ones128[:, :], 1.0)

            def emit_stage_b(h, part=0, tiles=None):
                # stage B: q_h [m, j] (token-major); qT via DMA xbar, then
                # cast to fp8 per 512-col chunk on alternating ScalarE/DVE.
                # For h>0 emitted in two halves (part 0: token tiles 0-7 +
                # casts c0/c1, part 1: tiles 8-15 + casts c2/c3) so each
                # half's qn copies drain within one job's DVE window.
                j0 = h * D
                if tiles is None:
                    qT = qt16_pool.tile([128, KT, N], bf16, name="qT", tag="qT")
                    qT8 = (qt8_pool.tile([128, KT, N], fp8, name="qT8",
                                         tag="qT8") if FP8_S else None)
                    qn = qn_pool.tile([128, NT, D], bf16, name="qn", tag="qn")
                else:
                    qT, qT8, qn = tiles

                def b_tile(mt):
                    ps = ps_stage.tile([128, D], f32, name="ps_b", tag="stage")
                    for k in range(KT):
                        nc.tensor.matmul(
                            ps[:, :],
                            lhsT=xT[:, k, mt * 128 : (mt + 1) * 128],
                            rhs=wq[:, k, j0 : j0 + D],
                            start=(k == 0),
                            stop=(k == KT - 1),
                        )
                    nc.vector.tensor_copy(qn[:, mt, :], ps[:, :])
                    # one xbar transpose per mt: [128, 512] -> [512, 128]
                    # scattered over the 4 j-tiles of qT (3D dest AP)
                    if h != 0 or mt >= CW // 128:
                        nc.sync.dma_start_transpose(
                            qT[:, :, mt * 128 : (mt + 1) * 128], qn[:, mt, :]
                        )

                def cast_chunk(c):
                    if not FP8_S:
                        return
                    # all casts on ScalarE: it is idle in the stage-B window
                    # (exps ran during the interleave), while DVE casts would
                    # delay the normalize-muls behind the qn copies
                    nc.scalar.copy(
                        qT8[:, :, c * CW : (c + 1) * CW],
                        qT[:, :, c * CW : (c + 1) * CW],
                    )

                if h == 0 and part == 0:
                    # head 0 has no prior work to hide the transpose latency
                    # behind: compute its first qT chunk directly on the PE,
                    # writing fp8 straight from PSUM via ScalarE.  Token
                    # tiles 12-15 are deferred to part 1, emitted mid-
                    # prologue to fill the PE idle behind the prologue exps.
                    for mt in range(4):
                        b_tile(mt)
                    for jt in range(KT):
                        ps = ps_stage.tile([128, CW], f32, name="ps_a", tag="stage")
                        for k in range(KT):
                            nc.tensor.matmul(
                                ps[:, :],
                                lhsT=wq[:, k, jt * 128 : (jt + 1) * 128],
                                rhs=xT[:, k, 0:CW],
                                start=(k == 0),
                                stop=(k == KT - 1),
                            )
                        nc.scalar.copy(
                            (qT8 if FP8_S else qT)[:, jt, 0:CW], ps[:, :]
                        )
                    for c in range(1, NCHUNK - 1):
                        for mt in range(4 * c, 4 * c + 4):
                            b_tile(mt)
                        cast_chunk(c)
                elif h == 0:
                    for mt in range(12, 16):
                        b_tile(mt)
                    cast_chunk(NCHUNK - 1)
                else:
                    for c in range(2 * part, 2 * part + 2):
                        for mt in range(4 * c, 4 * c + 4):
                            b_tile(mt)
                        cast_chunk(c)
                return (qT, qT8, qn)

            def emit_S(qT8s, c, mt, ps):
                # scores matmuls for (chunk c, token tile mt) into PSUM ps
                n0 = c * CW
                if FP8_S:
                    for kp in range(KT // 2):
                        nc.tensor.matmul(
                            ps[:, :],
                            lhsT=x8[:, 2 * kp : 2 * kp + 2,
                                    mt * 128 : (mt + 1) * 128],
                            rhs=qT8s[:, 2 * kp : 2 * kp + 2, n0 : n0 + CW],
                            start=(kp == 0),
                            stop=(kp == KT // 2 - 1),
                            perf_mode=DR,
                        )
                else:
                    for k in range(KT):
                        nc.tensor.matmul(
                            ps[:, :],
                            lhsT=xT[:, k, mt * 128 : (mt + 1) * 128],
                            rhs=qT8s[:, k, n0 : n0 + CW],
                            start=(k == 0),
                            stop=(k == KT - 1),
                        )

            # ---- software-pipelined main loop over 16 (head, chunk) jobs.
            # Job J's AV-dt0 matmuls interleave with job J+1's scores
            # matmuls so the PE never drains behind ScalarE's exp (600ns /
            # 512-col tile vs 434ns of fp8 scores matmul work). ----
            jobs = [(h, c) for h in range(HPG) for c in range(NCHUNK)]
            pending = emit_stage_b(0)
            _, qT8_cur, qn_cur = (pending[0], pending[1] or pending[0],
                                  pending[2])

            # prologue: job (0,0)'s scores, un-interleaved (overlaps the
            # input DMA waits at kernel start); head 0's deferred stage-B
            # tail fills the PE idle behind the prologue's exps
            expS_cur = exps_pool.tile([128, NT, CW], bf16, name="expS",
                                      tag="expS")
            for mt in range(NT):
                if mt == 8:
                    pending = emit_stage_b(0, part=1, tiles=pending)
                ps = ps_scores.tile([128, CW], f32, name="ps_s", tag="scores")
                emit_S(qT8_cur, 0, mt, ps)
                nc.scalar.activation(
                    expS_cur[:, mt, :], ps[:, :],
                    mybir.ActivationFunctionType.Exp, scale=INV_SQRT_D,
                )

            dtree_cur = dtree_pool.tile([128, 8, CW], bf16, name="dtree",
                                         tag="dtree")
            nc.vector.tensor_add(
                dtree_cur[:, 0:8, :], expS_cur[:, 0:8, :], expS_cur[:, 8:16, :]
            )
            nc.vector.tensor_add(
                dtree_cur[:, 0:4, :], dtree_cur[:, 0:4, :], dtree_cur[:, 4:8, :]
            )
            nc.vector.tensor_add(
                dtree_cur[:, 0:2, :], dtree_cur[:, 0:2, :], dtree_cur[:, 2:4, :]
            )
            nc.vector.tensor_add(
                dtree_cur[:, 0:1, :], dtree_cur[:, 0:1, :], dtree_cur[:, 1:2, :]
            )

            for idx, (h, c) in enumerate(jobs):
                nxt = jobs[idx + 1] if idx + 1 < len(jobs) else None
                if nxt is not None:
                    nh, ncc = nxt
                    if ncc == 0:
                        qT8_nxt = pending[1] if FP8_S else pending[0]
                        qn_nxt = pending[2]
                    else:
                        qT8_nxt, qn_nxt = qT8_cur, qn_cur
                    expS_nxt = exps_pool.tile([128, NT, CW], bf16,
                                              name="expS", tag="expS")

                # AV dt0 accumulation, interleaved with next job's scores
                outT = outt_pool.tile([128, KT, CW], bf16, name="outT",
                                      tag="outT")
                ps0 = ps_av.tile([128, CW], f32, name="ps_av", tag="av")
                for mt in range(NT):
                    if nxt is not None:
                        pss = ps_scores.tile([128, CW], f32, name="ps_s",
                                             tag="scores")
                        emit_S(qT8_nxt, ncc, mt, pss)
                        nc.scalar.activation(
                            expS_nxt[:, mt, :], pss[:, :],
                            mybir.ActivationFunctionType.Exp,
                            scale=INV_SQRT_D,
                        )
                    nc.tensor.matmul(
                        ps0[:, :],
                        lhsT=qn_cur[:, mt, 0:128],
                        rhs=expS_cur[:, mt, :],
                        start=(mt == 0),
                        stop=(mt == NT - 1),
                    )

                # single sum+broadcast matmul for the denominator (emitted
                # after the loop: earlier would head-of-line block the PE
                # on the DVE tree), then the reciprocal
                psb = ps_misc.tile([128, CW], f32, name="psb", tag="misc")
                nc.tensor.matmul(
                    psb[:, :], lhsT=ones128[:, :], rhs=dtree_cur[:, 0, :],
                    start=True, stop=True,
                )
                rcpB = rcp_pool.tile([128, CW], f32, name="rcpB", tag="rcpB")
                nc.vector.reciprocal_approx_fast(rcpB[:, :], psb[:, :])

                # next job's denominator tree, emitted now so it sits ahead
                # of stage-B's qn copies in the DVE queue
                if nxt is not None:
                    dtree_nxt = dtree_pool.tile([128, 8, CW], bf16,
                                                name="dtree", tag="dtree")
                    nc.vector.tensor_add(
                        dtree_nxt[:, 0:8, :], expS_nxt[:, 0:8, :],
                        expS_nxt[:, 8:16, :]
                    )
                    nc.vector.tensor_add(
                        dtree_nxt[:, 0:4, :], dtree_nxt[:, 0:4, :],
                        dtree_nxt[:, 4:8, :]
                    )
                    nc.vector.tensor_add(
                        dtree_nxt[:, 0:2, :], dtree_nxt[:, 0:2, :],
                        dtree_nxt[:, 2:4, :]
                    )
                    nc.vector.tensor_add(
                        dtree_nxt[:, 0:1, :], dtree_nxt[:, 0:1, :],
                        dtree_nxt[:, 1:2, :]
                    )

                # next head's stage B (full), before the dt1-3 block
                if c == NCHUNK - 2 and h + 1 < HPG:
                    pending = emit_stage_b(h + 1, part=0)
                    pending = emit_stage_b(h + 1, part=1, tiles=pending)

                # AV dt1..3; each dt's normalize-mul is emitted one dt late
                # so the reciprocal chain has a full dt of PE cover
                for dt in range(1, KT):
                    ps = ps_av.tile([128, CW], f32, name="ps_av", tag="av")
                    for mt in range(NT):
                        nc.tensor.matmul(
                            ps[:, :],
                            lhsT=qn_cur[:, mt, dt * 128 : (dt + 1) * 128],
                            rhs=expS_cur[:, mt, :],
                            start=(mt == 0),
                            stop=(mt == NT - 1),
                        )
                    prev = ps0 if dt == 1 else prev_ps
                    nc.vector.tensor_mul(
                        outT[:, dt - 1, :], prev[:, :], rcpB[:, :]
                    )
                    prev_ps = ps
                nc.vector.tensor_mul(
                    outT[:, KT - 1, :], prev_ps[:, :], rcpB[:, :]
                )

                # per-head projection for this chunk's n-tiles; host sums
                # the head partials
                def y_emit(ntl, dts, ps=None):
                    if ps is None:
                        ps = ps_stage.tile([128, D], f32, name="ps_y",
                                           tag="stage")
                    for dt in dts:
                        nc.tensor.matmul(
                            ps[:, :],
                            lhsT=outT[:, dt, ntl * 128 : (ntl + 1) * 128],
                            rhs=wp[:, h * KT + dt, :],
                            start=(dt == 0),
                            stop=(dt == KT - 1),
                        )
                    if dts[-1] != KT - 1:
                        return ps
                    nt = c * (CW // 128) + ntl
                    ysb = y_pool.tile([128, D], f32, name="ysb", tag="y")
                    if nt % 2 == 0:
                        nc.scalar.copy(ysb[:, :], ps[:, :])
                    else:
                        nc.vector.tensor_copy(ysb[:, :], ps[:, :])
                    nc.sync.dma_start(
                        y_d[h * N + nt * 128 : h * N + (nt + 1) * 128, :],
                        ysb[:, :],
                    )
                    return None

                # first two n-tiles defer their dt3 matmul so the PE isn't
                # waiting on the last normalize-mul right after dt3's
                # accumulation finishes
                p0 = y_emit(0, [0, 1, 2])
                p1 = y_emit(1, [0, 1, 2])
                y_emit(0, [3], ps=p0)
                y_emit(1, [3], ps=p1)
                y_emit(2, [0, 1, 2, 3])
                y_emit(3, [0, 1, 2, 3])

                if nxt is not None:
                    expS_cur = expS_nxt
                    dtree_cur = dtree_nxt
                    qT8_cur, qn_cur = qT8_nxt, qn_nxt

    nc.compile()
    return nc


def _ensure_nc():
    if "nc" not in _state:
        _state["nc"] = _build()
    return _state["nc"]


def _make_in_maps(x, Wq, Wp):
    bf = np.float16
    f8 = ml_dtypes.float8_e4m3
    in_maps = []
    for c in range(NCORES):
        b, hg = c // HG, c % HG
        xt = np.ascontiguousarray(x[b].T)
        m = {
            "xt": xt.astype(bf),
            "wq": np.ascontiguousarray(Wq[:, hg * JW : (hg + 1) * JW]).astype(bf),
            "wp": np.ascontiguousarray(Wp[hg * JW : (hg + 1) * JW, :]).astype(bf),
        }
        if FP8_S:
            m["xt8"] = xt.astype(f8)
        in_maps.append(m)
    return in_maps


def _get_runner():
    """Build once and cache a jitted 8-core runner (avoids re-jit per call)."""
    if "run" in _state:
        return _state["run"]

    import jax
    import concourse.mybir as mybir
    from jax.sharding import Mesh, PartitionSpec
    from jax.experimental.shard_map import shard_map
    from concourse import bass2jax

    nc = _ensure_nc()
    bass2jax.install_neuronx_cc_hook()

    partition_name = nc.partition_id_tensor.name if nc.partition_id_tensor else None
    in_names, out_names, out_avals, zero_outs = [], [], [], []
    for alloc in nc.m.functions[0].allocations:
        if not isinstance(alloc, mybir.MemoryLocationSet):
            continue
        name = alloc.memorylocations[0].name
        if alloc.kind == "ExternalInput":
            if name != partition_name:
                in_names.append(name)
        elif alloc.kind == "ExternalOutput":
            shape = tuple(alloc.tensor_shape)
            dtype = mybir.dt.np(alloc.dtype)
            out_avals.append(jax.core.ShapedArray(shape, dtype))
            out_names.append(name)
            zero_outs.append(np.zeros(shape, dtype))
    n_params = len(in_names)
    n_outs = len(out_names)
    all_in_names = list(in_names) + list(out_names)
    if partition_name is not None:
        all_in_names.append(partition_name)

    def _body(*args):
        operands = list(args)
        if partition_name is not None:
            operands.append(bass2jax.partition_id_tensor())
        outs = bass2jax._bass_exec_p.bind(
            *operands,
            out_avals=tuple(out_avals),
            in_names=tuple(all_in_names),
            out_names=tuple(out_names),
            lowering_input_output_aliases=(),
            sim_require_finite=True,
            sim_require_nnan=True,
            nc=nc,
        )
        return tuple(outs)

    devices = jax.devices()[:NCORES]
    mesh = Mesh(np.asarray(devices), ("core",))
    in_specs = (PartitionSpec("core"),) * (n_params + n_outs)
    out_specs = (PartitionSpec("core"),) * n_outs
    sharded = jax.jit(
        shard_map(_body, mesh=mesh, in_specs=in_specs, out_specs=out_specs,
                  check_rep=False),
        donate_argnums=tuple(range(n_params, n_params + n_outs)),
        keep_unused=True,
    )

    def run(in_maps):
        concat_in = [
            np.concatenate([np.asarray(m[name]) for m in in_maps], axis=0)
            for name in in_names
        ]
        concat_zeros = [
            np.zeros((NCORES * z.shape[0], *z.shape[1:]), z.dtype) for z in zero_outs
        ]
        out_arrs = sharded(*concat_in, *concat_zeros)
        return [
            {
                name: np.asarray(out_arrs[i]).reshape(NCORES, *out_avals[i].shape)[c]
                for i, name in enumerate(out_names)
            }
            for c in range(NCORES)
        ]

    _state["run"] = run
    return run


def kernel(x, Wq, Wv, Wp, bp):
    x = np.asarray(x, np.float32)
    Wq = np.asarray(Wq, np.float32)
    Wp = np.asarray(Wp, np.float32)
    bp = np.asarray(bp, np.float32)

    run = _get_runner()
    results = run(_make_in_maps(x, Wq, Wp))
    y = np.empty((B, N, D), np.float32)
    for b in range(B):
        y[b] = (results[b * HG]["y"].reshape(HPG, N, D).sum(axis=0)
                + results[b * HG + 1]["y"].reshape(HPG, N, D).sum(axis=0)
                + bp[None, :])
    return y


# revision 29
# speedup vs baseline: 1.0418x; 1.0029x over previous
"""Multi-head attention V2 kernel for Trainium2 (8 NeuronCores).

Problem shapes (hardcoded): x [4, 2048, 512] f32, Wq [512, 4096], Wv unused,
Wp [4096, 512], bp [512].  Reference math (note: V uses the Q projection):
    q = v = (x @ Wq) -> [B, H, N, D] with H=8, head dim = D = 512
    S = q @ x^T / sqrt(D);  P = softmax(S, -1);  out = (P @ v) @ Wp + bp

Sharding: core = (batch b, head-group hg) with 2 groups of 4 heads.
Each core gets x[b]^T and the Wq columns / Wp rows of its 4 heads, computes
its per-head partial outputs [HPG, N, D]; host sums the 8 head partials per
batch and adds the bias.

Per-core kernel (software-pipelined over 16 (head, chunk) jobs):
  xT [512, 2048] fp16 + fp8e4m3, Wq fp16, Wp fp16 resident in SBUF.
  Stage B per head (emitted during the previous head's jobs):
    q [m, j] = x Wq_h  (fp16 matmuls, fp32 PSUM) -> SBUF fp16 qn
    qT[j, n] = q_h^T via DMA xbar transposes, cast fp16 -> fp8e4m3
               (alternating ScalarE/DVE per 512-column chunk)
  Per job J = (head h, 512-column chunk c):
    S^T[m, n] of job J+1 = x q^T as fp8e4m3 DoubleRow matmuls (2 passes
      of K=256, lhsT [128,2,128] / rhs [128,2,512], same rows/cycle as
      fp16 -> 2x FLOP rate), INTERLEAVED with job J's AV-dt0 matmuls so
      the PE (647ns/iter) outpaces ScalarE's exp (667ns) without
      stalling on the scores-PSUM ring.  fp8 scores cost ~1.9e-2 global
      rel err (fp16 elsewhere keeps the rest at ~4e-4), under the 2e-2
      gate.  expS = exp(S^T/sqrt(D)) on ScalarE, PSUM -> SBUF fp16.
    den: DVE pairwise tree-add of J's 16 expS tiles (4 wide fp16 adds,
      emitted ahead of stage-B's qn copies in the DVE queue), then ONE
      all-ones [128,128] fp16 matmul that simultaneously sums over the
      128 partitions and broadcasts den to all partitions.
    rcpB = 1/den (DVE reciprocal_approx_fast); U^T = q^T expS (fp16
      matmuls, lhsT=qn); outT = U^T * rcpB (DVE, muls staggered one dt
      late so the reciprocal chain has PE cover).
    y_h[n, e] = sum_dt outT[dt, n-tile]^T Wp[h*4+dt] -> HBM f32 partial
      per head; the host sums the 8 per-head partials per batch + bias.
Softmax skips the max-subtraction: scores are q.x/sqrt(512) with |s| < ~6,
so exp is safely in fp32 range and the result is mathematically identical.
"""

import os
import sys

sys.path.insert(0, "/opt/trn_rl_repo")

import numpy as np
import ml_dtypes

# fp8 DoubleRow scores toggle (KFP8=0 falls back to fp16 scores, for
# debugging accuracy vs speed)
FP8_S = os.environ.get("KFP8", "1") == "1"

B, N, D, H = 4, 2048, 512, 8
NCORES = 8
HG = 2            # head groups (cores per batch)
HPG = H // HG     # heads per core
JW = HPG * D      # per-core Wq column count / Wp row count (2048)
KT = D // 128     # k-tiles over feature dim (4)
NT = N // 128     # partition tiles over tokens (16)
NCHUNK = 4        # n split into 4 chunks of 512
CW = N // NCHUNK  # chunk width (512)
INV_SQRT_D = 1.0 / float(np.sqrt(D))

_state = {}


def _build():
    import concourse.bass as bass
    import concourse.mybir as mybir
    import concourse.tile as tile
    from concourse import bacc

    f32 = mybir.dt.float32
    bf16 = mybir.dt.float16
    fp8 = mybir.dt.float8e4
    DR = mybir.MatmulPerfMode.DoubleRow

    nc = bacc.Bacc("TRN2", target_bir_lowering=False)

    xT_d = nc.dram_tensor("xt", [D, N], bf16, kind="ExternalInput")
    x8_d = (nc.dram_tensor("xt8", [D, N], fp8, kind="ExternalInput")
            if FP8_S else None)
    wq_d = nc.dram_tensor("wq", [D, JW], bf16, kind="ExternalInput")
    wp_d = nc.dram_tensor("wp", [JW, D], bf16, kind="ExternalInput")
    y_d = nc.dram_tensor("y", [HPG * N, D], f32, kind="ExternalOutput")

    with tile.TileContext(nc) as tc:
        with (
            tc.tile_pool(name="const", bufs=1) as cpool,
            tc.tile_pool(name="qt16", bufs=1) as qt16_pool,
            tc.tile_pool(name="qt8", bufs=2) as qt8_pool,
            tc.tile_pool(name="qn", bufs=2) as qn_pool,
            tc.tile_pool(name="exps", bufs=2) as exps_pool,
            tc.tile_pool(name="dtree", bufs=2) as dtree_pool,
            tc.tile_pool(name="outt", bufs=2) as outt_pool,
            tc.tile_pool(name="rcp", bufs=2) as rcp_pool,
            tc.tile_pool(name="ysb", bufs=3) as y_pool,
            tc.tile_pool(name="ps_stage", bufs=2, space="PSUM") as ps_stage,
            tc.tile_pool(name="ps_scores", bufs=3, space="PSUM") as ps_scores,
            tc.tile_pool(name="ps_av", bufs=2, space="PSUM") as ps_av,
            tc.tile_pool(name="ps_misc", bufs=1, space="PSUM") as ps_misc,
        ):
            # ---- resident inputs ----
            xT = cpool.tile([128, KT, N], bf16, name="xT")
            x8 = (cpool.tile([128, KT, N], fp8, name="x8")
                  if FP8_S else None)
            wq = cpool.tile([128, KT, JW], bf16, name="wq")
            wp = cpool.tile([128, JW // 128, D], bf16, name="wp")
            # critical first wave, finest first: the very first stage-B
            # matmul group needs only xT cols 0:128 of each k-tile plus the
            # head-0 Wq block (~640KB), so land those before the rest
            for k in range(KT):
                nc.sync.dma_start(
                    xT[:, k, 0:128], xT_d[k * 128 : (k + 1) * 128, 0:128]
                )
                # split the head-0 wq tiles so no single queue serializes
                # the very first stage-B matmul group
                for w in range(2):
                    nc.sync.dma_start(
                        wq[:, k, w * 256 : (w + 1) * 256],
                        wq_d[k * 128 : (k + 1) * 128, w * 256 : (w + 1) * 256],
                    )
            for k in range(KT):
                nc.sync.dma_start(
                    xT[:, k, 128:CW], xT_d[k * 128 : (k + 1) * 128, 128:CW]
                )
            for k in range(KT):
                nc.sync.dma_start(
                    xT[:, k, CW:N], xT_d[k * 128 : (k + 1) * 128, CW:N]
                )
            # fp8 copy of x^T: first consumed by head-0 chunk-0 scores,
            # ~25us in; lands after the fp16 critical waves
            if FP8_S:
                for k in range(KT):
                    nc.sync.dma_start(
                        x8[:, k, :], x8_d[k * 128 : (k + 1) * 128, :]
                    )

            def load_noncritical():
                # wq for heads 1-3 (first needed ~100us in) and wp (first
                # needed by head-0 chunk-0 projection): emitted after head
                # 0's transposes so the critical wave gets full bandwidth
                for h in range(1, HPG):
                    for k in range(KT):
                        nc.sync.dma_start(
                            wq[:, k, h * D : (h + 1) * D],
                            wq_d[k * 128 : (k + 1) * 128, h * D : (h + 1) * D],
                        )
                for j in range(JW // 128):
                    nc.sync.dma_start(wp[:, j, :], wp_d[j * 128 : (j + 1) * 128, :])

            load_noncritical()

            ones_col = cpool.tile([128, 1], bf16, name="ones_col")
            nc.vector.memset(ones_col[:, :], 1.0)
            # touch Exp once during the input-DMA wait so the ~2.7us ACT
            # table-set load is off the first chunk's critical path
            nc.scalar.activation(
                ones_col[:, :], ones_col[:, :],
                mybir.ActivationFunctionType.Exp, scale=0.0,
            )
            nc.vector.memset(ones_col[:, :], 1.0)
            # all-ones [128,128] fp16: one matmul against the tree-reduced
            # dtree row block both sums over partitions and broadcasts the
            # result to all 128 output partitions
            ones128 = cpool.tile([128, 128], bf16, name="ones128")
            nc.vector.memset(ones128[:, :], 1.0)

            def emit_stage_b(h, part=0, tiles=None):
                # stage B: q_h [m, j] (token-major); qT via DMA xbar, then
                # cast to fp8 per 512-col chunk on alternating ScalarE/DVE.
                # For h>0 emitted in two halves (part 0: token tiles 0-7 +
                # casts c0/c1, part 1: tiles 8-15 + casts c2/c3) so each
                # half's qn copies drain within one job's DVE window.
                j0 = h * D
                if tiles is None:
                    qT = qt16_pool.tile([128, KT, N], bf16, name="qT", tag="qT")
                    qT8 = (qt8_pool.tile([128, KT, N], fp8, name="qT8",
                                         tag="qT8") if FP8_S else None)
                    qn = qn_pool.tile([128, NT, D], bf16, name="qn", tag="qn")
                else:
                    qT, qT8, qn = tiles

                def b_tile(mt):
                    ps = ps_stage.tile([128, D], f32, name="ps_b", tag="stage")
                    for k in range(KT):
                        nc.tensor.matmul(
                            ps[:, :],
                            lhsT=xT[:, k, mt * 128 : (mt + 1) * 128],
                            rhs=wq[:, k, j0 : j0 + D],
                            start=(k == 0),
                            stop=(k == KT - 1),
                        )
                    nc.vector.tensor_copy(qn[:, mt, :], ps[:, :])
                    # one xbar transpose per mt: [128, 512] -> [512, 128]
                    # scattered over the 4 j-tiles of qT (3D dest AP)
                    if h != 0 or mt >= CW // 128:
                        nc.sync.dma_start_transpose(
                            qT[:, :, mt * 128 : (mt + 1) * 128], qn[:, mt, :]
                        )

                def cast_chunk(c):
                    if not FP8_S:
                        return
                    # all casts on ScalarE: it is idle in the stage-B window
                    # (exps ran during the interleave), while DVE casts would
                    # delay the normalize-muls behind the qn copies
                    nc.scalar.copy(
                        qT8[:, :, c * CW : (c + 1) * CW],
                        qT[:, :, c * CW : (c + 1) * CW],
                    )

                if h == 0 and part == 0:
                    # head 0 has no prior work to hide the transpose latency
                    # behind: compute its first qT chunk directly on the PE,
                    # writing fp8 straight from PSUM via ScalarE.  Token
                    # tiles 12-15 are deferred to part 1, emitted mid-
                    # prologue to fill the PE idle behind the prologue exps.
                    for mt in range(4):
                        b_tile(mt)
                    for jt in range(KT):
                        ps = ps_stage.tile([128, CW], f32, name="ps_a", tag="stage")
                        for k in range(KT):
                            nc.tensor.matmul(
                                ps[:, :],
                                lhsT=wq[:, k, jt * 128 : (jt + 1) * 128],
                                rhs=xT[:, k, 0:CW],
                                start=(k == 0),
                                stop=(k == KT - 1),
                            )
                        nc.scalar.copy(
                            (qT8 if FP8_S else qT)[:, jt, 0:CW], ps[:, :]
                        )
                    for c in range(1, NCHUNK - 1):
                        for mt in range(4 * c, 4 * c + 4):
                            b_tile(mt)
                        cast_chunk(c)
                elif h == 0:
                    for mt in range(12, 16):
                        b_tile(mt)
                    cast_chunk(NCHUNK - 1)
                else:
                    for c in range(2 * part, 2 * part + 2):
                        for mt in range(4 * c, 4 * c + 4):
                            b_tile(mt)
                        cast_chunk(c)
                return (qT, qT8, qn)

            def emit_S(qT8s, c, mt, ps):
                # scores matmuls for (chunk c, token tile mt) into PSUM ps
                n0 = c * CW
                if FP8_S:
                    for kp in range(KT // 2):
                        nc.tensor.matmul(
                            ps[:, :],
                            lhsT=x8[:, 2 * kp : 2 * kp + 2,
                                    mt * 128 : (mt + 1) * 128],
                            rhs=qT8s[:, 2 * kp : 2 * kp + 2, n0 : n0 + CW],
                            start=(kp == 0),
                            stop=(kp == KT // 2 - 1),
                            perf_mode=DR,
                        )
                else:
                    for k in range(KT):
                        nc.tensor.matmul(
                            ps[:, :],
                            lhsT=xT[:, k, mt * 128 : (mt + 1) * 128],
                            rhs=qT8s[:, k, n0 : n0 + CW],
                            start=(k == 0),
                            stop=(k == KT - 1),
                        )

            # ---- software-pipelined main loop over 16 (head, chunk) jobs.
            # Job J's AV-dt0 matmuls interleave with job J+1's scores
            # matmuls so the PE never drains behind ScalarE's exp (600ns /
            # 512-col tile vs 434ns of fp8 scores matmul work). ----
            jobs = [(h, c) for h in range(HPG) for c in range(NCHUNK)]
            pending = emit_stage_b(0)
            _, qT8_cur, qn_cur = (pending[0], pending[1] or pending[0],
                                  pending[2])

            # prologue: job (0,0)'s scores, un-interleaved (overlaps the
            # input DMA waits at kernel start); head 0's deferred stage-B
            # tail fills the PE idle behind the prologue's exps
            expS_cur = exps_pool.tile([128, NT, CW], bf16, name="expS",
                                      tag="expS")
            for mt in range(NT):
                if mt == 8:
                    pending = emit_stage_b(0, part=1, tiles=pending)
                ps = ps_scores.tile([128, CW], f32, name="ps_s", tag="scores")
                emit_S(qT8_cur, 0, mt, ps)
                nc.scalar.activation(
                    expS_cur[:, mt, :], ps[:, :],
                    mybir.ActivationFunctionType.Exp, scale=INV_SQRT_D,
                )

            dtree_cur = dtree_pool.tile([128, 8, CW], bf16, name="dtree",
                                         tag="dtree")
            nc.vector.tensor_add(
                dtree_cur[:, 0:8, :], expS_cur[:, 0:8, :], expS_cur[:, 8:16, :]
            )
            nc.vector.tensor_add(
                dtree_cur[:, 0:4, :], dtree_cur[:, 0:4, :], dtree_cur[:, 4:8, :]
            )
            nc.vector.tensor_add(
                dtree_cur[:, 0:2, :], dtree_cur[:, 0:2, :], dtree_cur[:, 2:4, :]
            )
            nc.vector.tensor_add(
                dtree_cur[:, 0:1, :], dtree_cur[:, 0:1, :], dtree_cur[:, 1:2, :]
            )

            for idx, (h, c) in enumerate(jobs):
                nxt = jobs[idx + 1] if idx + 1 < len(jobs) else None
                if nxt is not None:
                    nh, ncc = nxt
                    if ncc == 0:
                        qT8_nxt = pending[1] if FP8_S else pending[0]
                        qn_nxt = pending[2]
                    else:
                        qT8_nxt, qn_nxt = qT8_cur, qn_cur
                    expS_nxt = exps_pool.tile([128, NT, CW], bf16,
                                              name="expS", tag="expS")

                # AV dt0 accumulation, interleaved with next job's scores
                outT = outt_pool.tile([128, KT, CW], bf16, name="outT",
                                      tag="outT")
                ps0 = ps_av.tile([128, CW], f32, name="ps_av", tag="av")
                for mt in range(NT):
                    if nxt is not None:
                        pss = ps_scores.tile([128, CW], f32, name="ps_s",
                                             tag="scores")
                        emit_S(qT8_nxt, ncc, mt, pss)
                        nc.scalar.activation(
                            expS_nxt[:, mt, :], pss[:, :],
                            mybir.ActivationFunctionType.Exp,
                            scale=INV_SQRT_D,
                        )
                    nc.tensor.matmul(
                        ps0[:, :],
                        lhsT=qn_cur[:, mt, 0:128],
                        rhs=expS_cur[:, mt, :],
                        start=(mt == 0),
                        stop=(mt == NT - 1),
                    )

                # single sum+broadcast matmul for the denominator (emitted
                # after the loop: earlier would head-of-line block the PE
                # on the DVE tree), then the reciprocal
                psb = ps_misc.tile([128, CW], f32, name="psb", tag="misc")
                nc.tensor.matmul(
                    psb[:, :], lhsT=ones128[:, :], rhs=dtree_cur[:, 0, :],
                    start=True, stop=True,
                )
                rcpB = rcp_pool.tile([128, CW], f32, name="rcpB", tag="rcpB")
                nc.vector.reciprocal_approx_fast(rcpB[:, :], psb[:, :])

                # next head's stage B (full), before the dt1-3 block
                if c == NCHUNK - 2 and h + 1 < HPG:
                    pending = emit_stage_b(h + 1, part=0)
                    pending = emit_stage_b(h + 1, part=1, tiles=pending)

                # AV dt1..3; each dt's normalize-mul is emitted one dt late
                # so the reciprocal chain has a full dt of PE cover
                for dt in range(1, KT):
                    ps = ps_av.tile([128, CW], f32, name="ps_av", tag="av")
                    for mt in range(NT):
                        nc.tensor.matmul(
                            ps[:, :],
                            lhsT=qn_cur[:, mt, dt * 128 : (dt + 1) * 128],
                            rhs=expS_cur[:, mt, :],
                            start=(mt == 0),
                            stop=(mt == NT - 1),
                        )
                    prev = ps0 if dt == 1 else prev_ps
                    nc.vector.tensor_mul(
                        outT[:, dt - 1, :], prev[:, :], rcpB[:, :]
                    )
                    prev_ps = ps
                nc.vector.tensor_mul(
                    outT[:, KT - 1, :], prev_ps[:, :], rcpB[:, :]
                )

                # next job's denominator tree: emitted after the muls (so
                # mul(dt0) isn't delayed behind 4us of adds in the DVE
                # queue) but still ahead of stage-B's qn copies; finishes
                # well before job J+1's broadcast matmul reads it
                if nxt is not None:
                    dtree_nxt = dtree_pool.tile([128, 8, CW], bf16,
                                                name="dtree", tag="dtree")
                    nc.vector.tensor_add(
                        dtree_nxt[:, 0:8, :], expS_nxt[:, 0:8, :],
                        expS_nxt[:, 8:16, :]
                    )
                    nc.vector.tensor_add(
                        dtree_nxt[:, 0:4, :], dtree_nxt[:, 0:4, :],
                        dtree_nxt[:, 4:8, :]
                    )
                    nc.vector.tensor_add(
                        dtree_nxt[:, 0:2, :], dtree_nxt[:, 0:2, :],
                        dtree_nxt[:, 2:4, :]
                    )
                    nc.vector.tensor_add(
                        dtree_nxt[:, 0:1, :], dtree_nxt[:, 0:1, :],
                        dtree_nxt[:, 1:2, :]
                    )

                # per-head projection for this chunk's n-tiles; host sums
                # the head partials
                def y_emit(ntl, dts, ps=None):
                    if ps is None:
                        ps = ps_stage.tile([128, D], f32, name="ps_y",
                                           tag="stage")
                    for dt in dts:
                        nc.tensor.matmul(
                            ps[:, :],
                            lhsT=outT[:, dt, ntl * 128 : (ntl + 1) * 128],
                            rhs=wp[:, h * KT + dt, :],
                            start=(dt == 0),
                            stop=(dt == KT - 1),
                        )
                    if dts[-1] != KT - 1:
                        return ps
                    nt = c * (CW // 128) + ntl
                    ysb = y_pool.tile([128, D], f32, name="ysb", tag="y")
                    if nt % 2 == 0:
                        nc.scalar.copy(ysb[:, :], ps[:, :])
                    else:
                        nc.vector.tensor_copy(ysb[:, :], ps[:, :])
                    nc.sync.dma_start(
                        y_d[h * N + nt * 128 : h * N + (nt + 1) * 128, :],
                        ysb[:, :],
                    )
                    return None

                # first two n-tiles defer their dt3 matmul so the PE isn't
                # waiting on the last normalize-mul right after dt3's
                # accumulation finishes
                p0 = y_emit(0, [0, 1, 2])
                p1 = y_emit(1, [0, 1, 2])
                y_emit(0, [3], ps=p0)
                y_emit(1, [3], ps=p1)
                y_emit(2, [0, 1, 2, 3])
                y_emit(3, [0, 1, 2, 3])

                if nxt is not None:
                    expS_cur = expS_nxt
                    dtree_cur = dtree_nxt
                    qT8_cur, qn_cur = qT8_nxt, qn_nxt

    nc.compile()
    return nc


def _ensure_nc():
    if "nc" not in _state:
        _state["nc"] = _build()
    return _state["nc"]


def _make_in_maps(x, Wq, Wp):
    bf = np.float16
    f8 = ml_dtypes.float8_e4m3
    in_maps = []
    for c in range(NCORES):
        b, hg = c // HG, c % HG
        xt = np.ascontiguousarray(x[b].T)
        m = {
            "xt": xt.astype(bf),
            "wq": np.ascontiguousarray(Wq[:, hg * JW : (hg + 1) * JW]).astype(bf),
            "wp": np.ascontiguousarray(Wp[hg * JW : (hg + 1) * JW, :]).astype(bf),
        }
        if FP8_S:
            m["xt8"] = xt.astype(f8)
        in_maps.append(m)
    return in_maps


def _get_runner():
    """Build once and cache a jitted 8-core runner (avoids re-jit per call)."""
    if "run" in _state:
        return _state["run"]

    import jax
    import concourse.mybir as mybir
    from jax.sharding import Mesh, PartitionSpec
    from jax.experimental.shard_map import shard_map
    from concourse import bass2jax

    nc = _ensure_nc()
    bass2jax.install_neuronx_cc_hook()

    partition_name = nc.partition_id_tensor.name if nc.partition_id_tensor else None
    in_names, out_names, out_avals, zero_outs = [], [], [], []
    for alloc in nc.m.functions[0].allocations:
        if not isinstance(alloc, mybir.MemoryLocationSet):
            continue
        name = alloc.memorylocations[0].name
        if alloc.kind == "ExternalInput":
            if name != partition_name:
                in_names.append(name)
        elif alloc.kind == "ExternalOutput":
            shape = tuple(alloc.tensor_shape)
            dtype = mybir.dt.np(alloc.dtype)
            out_avals.append(jax.core.ShapedArray(shape, dtype))
            out_names.append(name)
            zero_outs.append(np.zeros(shape, dtype))
    n_params = len(in_names)
    n_outs = len(out_names)
    all_in_names = list(in_names) + list(out_names)
    if partition_name is not None:
        all_in_names.append(partition_name)

    def _body(*args):
        operands = list(args)
        if partition_name is not None:
            operands.append(bass2jax.partition_id_tensor())
        outs = bass2jax._bass_exec_p.bind(
            *operands,
            out_avals=tuple(out_avals),
            in_names=tuple(all_in_names),
            out_names=tuple(out_names),
            lowering_input_output_aliases=(),
            sim_require_finite=True,
            sim_require_nnan=True,
            nc=nc,
        )
        return tuple(outs)

    devices = jax.devices()[:NCORES]
    mesh = Mesh(np.asarray(devices), ("core",))
    in_specs = (PartitionSpec("core"),) * (n_params + n_outs)
    out_specs = (PartitionSpec("core"),) * n_outs
    sharded = jax.jit(
        shard_map(_body, mesh=mesh, in_specs=in_specs, out_specs=out_specs,
                  check_rep=False),
        donate_argnums=tuple(range(n_params, n_params + n_outs)),
        keep_unused=True,
    )

    def run(in_maps):
        concat_in = [
            np.concatenate([np.asarray(m[name]) for m in in_maps], axis=0)
            for name in in_names
        ]
        concat_zeros = [
            np.zeros((NCORES * z.shape[0], *z.shape[1:]), z.dtype) for z in zero_outs
        ]
        out_arrs = sharded(*concat_in, *concat_zeros)
        return [
            {
                name: np.asarray(out_arrs[i]).reshape(NCORES, *out_avals[i].shape)[c]
                for i, name in enumerate(out_names)
            }
            for c in range(NCORES)
        ]

    _state["run"] = run
    return run


def kernel(x, Wq, Wv, Wp, bp):
    x = np.asarray(x, np.float32)
    Wq = np.asarray(Wq, np.float32)
    Wp = np.asarray(Wp, np.float32)
    bp = np.asarray(bp, np.float32)

    run = _get_runner()
    results = run(_make_in_maps(x, Wq, Wp))
    y = np.empty((B, N, D), np.float32)
    for b in range(B):
        y[b] = (results[b * HG]["y"].reshape(HPG, N, D).sum(axis=0)
                + results[b * HG + 1]["y"].reshape(HPG, N, D).sum(axis=0)
                + bp[None, :])
    return y
